# revision 28
# baseline (speedup 1.0000x reference)
"""Baichuan attention prefill on 8 TRN2 NeuronCores.

Tensor-parallel over heads: 5 heads per core. Per core:
  QKV projection (its heads' slice of W_pack) -> attention (causal,
  softmax without max-subtraction) -> AllGather of attention output
  O^T across cores -> o_proj with full contraction producing this
  core's 640 output dims. Host assembles the [1, 2048, 5120] output.

v6: fine-grained interleaving. Attention chunk ci's tiles are emitted
interspersed with QKV quarter ci+1's PSUM groups (and o_proj groups
under the last chunks), so the scalar-engine exp latency - the real
attention ceiling - is hidden under PE matmul work that does not
depend on it. Softmax denominators come from a DVE-accumulated sum of
the exp tiles followed by a single ones-matmul per (chunk, head)
instead of a ones-matmul per key tile, cutting the PE L-pass ~10x.
Reciprocal runs on the scalar engine. One set of PSUM pools with
shared tags across all phases keeps the PE queue barrier-free.
"""

import numpy as np
import ml_dtypes

import concourse.bacc as bacc
import concourse.mybir as mybir
from concourse.tile import TileContext
from concourse.bass_utils import run_bass_kernel_spmd

HID = 5120
NH = 40
HD = 128
S = 2048
N_CORES = 8
HPC = NH // N_CORES          # 5 heads per core
DPC = HPC * HD               # 640 dims per core
F32 = mybir.dt.float32
BF16 = mybir.dt.bfloat16
SCALE = 1.0 / float(np.sqrt(HD))

SB = 1024                    # hid superblock (8 x 128 subtiles)
NSB = HID // SB              # 5 superblocks
NST = SB // 128              # 8 subtiles per superblock
SEQ_BLK = 512                # QKV seq quarter
NQ = S // SEQ_BLK            # 4
# attention / AllGather chunks (q0, qlen); finer at the end so the last
# AllGather lands early and o_proj can finish right behind it
CHUNKS = [(0, 512), (512, 512), (1024, 512), (1536, 256), (1792, 256)]
OSUB = 256                   # o_proj column sub-chunk
RG = [list(range(N_CORES))]

_graph_cache = None


def _scalar_reciprocal(nc, out, in_):
    """Reciprocal on the Activation engine (~0.7us vs ~4us on DVE for a
    [128,512] tile). The softmax denominator tolerates activation-table
    accuracy (verified against the fp64 oracle)."""
    eng = nc.scalar
    ins = [eng.lower_ap(in_)]
    for v in (0.0, 1.0, 0.0):
        ins.append(mybir.ImmediateValue(dtype=mybir.dt.float32, value=v))
    return eng.add_instruction(
        mybir.InstActivation(
            name=eng.bass.get_next_instruction_name(),
            func=mybir.ActivationFunctionType.Reciprocal,
            ins=ins,
            outs=[eng.lower_ap(out)],
        )
    )


def _interleave(primary, inserts, ratio, per=1):
    """Drive two emission generators: advance `primary` and insert `per`
    steps of `inserts` every `ratio` primary steps. Drains both."""
    i = 0
    for _ in primary:
        i += 1
        if inserts is not None and i % ratio == 0:
            for _ in range(per):
                next(inserts, None)
    if inserts is not None:
        for _ in inserts:
            pass


def _build_graph():
    nc = bacc.Bacc(name="baichuan_attn")

    xt = nc.declare_dram_parameter("xt", [HID, S], BF16, isOutput=False)
    wqt = nc.declare_dram_parameter("wqt", [HID, DPC], BF16, isOutput=False)
    wkt = nc.declare_dram_parameter("wkt", [HID, DPC], BF16, isOutput=False)
    wvt = nc.declare_dram_parameter("wvt", [HID, DPC], BF16, isOutput=False)
    # wot_t[mt][p][ht*128+c] = o_proj_w[640c_core + 128 mt + c, 128 ht + p]
    wot_t = nc.declare_dram_parameter("wot_t", [HPC, 128, HID], BF16, isOutput=False)
    # {0,1} causal masks for diagonal key tiles, by relative tile offset
    dmask = nc.declare_dram_parameter("dmask", [4, 128, 512], BF16, isOutput=False)
    ones = nc.declare_dram_parameter("ones", [128, 128], BF16, isOutput=False)
    out = nc.declare_dram_parameter("out", [DPC, S], F32, isOutput=True)

    warm_in = nc.dram_tensor("warm_in", [128, 8], BF16)
    warm_out = nc.dram_tensor("warm_out", [1024, 8], BF16, addr_space="Shared")
    ot_b = [
        nc.dram_tensor(f"ot_b{ci}", [DPC, qlen], BF16)
        for ci, (q0, qlen) in enumerate(CHUNKS)
    ]
    og = [
        nc.dram_tensor(f"og{ci}", [HID, qlen], BF16, addr_space="Shared")
        for ci, (q0, qlen) in enumerate(CHUNKS)
    ]

    with TileContext(nc) as tc:
        nc.gpsimd.collective_compute(
            "AllGather",
            mybir.AluOpType.bypass,
            replica_groups=RG,
            ins=[warm_in.ap().opt()],
            outs=[warm_out.ap().opt()],
        )
        with (
            # PSUM pools shared by every phase (same tags) so no pool
            # boundary ever blocks the PE queue: 4+2+2 = 8 banks.
            tc.tile_pool(name="psA", bufs=4, space="PSUM") as psA,
            tc.tile_pool(name="psB", bufs=2, space="PSUM") as psB,
            tc.tile_pool(name="psD", bufs=2, space="PSUM") as psD,
            tc.tile_pool(name="kv_sb", bufs=1) as kvp,
            tc.tile_pool(name="q_sb", bufs=2) as qp,
            tc.tile_pool(name="cst", bufs=1) as cstp,
            tc.tile_pool(name="at_p", bufs=6) as pp,
            tc.tile_pool(name="at_o", bufs=1) as op_,
            tc.tile_pool(name="op_y", bufs=3) as yp,
            tc.tile_pool(name="op_w", bufs=1) as wcp,
        ):
            k_q = [
                kvp.tile([128, HPC, SEQ_BLK], BF16, name=f"k_sb{qq}")
                for qq in range(NQ)
            ]
            v_q = [
                kvp.tile([128, SEQ_BLK // 128, DPC], BF16, name=f"v_sb{qq}")
                for qq in range(NQ)
            ]
            mask_sb = cstp.tile([128, 4, 512], BF16, name="mask_sb")
            ones_sb = cstp.tile([128, 128], BF16, name="ones_sb")

            pools = (psA, psB, psD, pp, op_)
            q_tiles = {}

            def attn_gen(ci):
                return _attn_chunk_gen(
                    nc, ci, q_tiles[CHUNKS[ci][0] // SEQ_BLK], k_q, v_q,
                    mask_sb, ones_sb, pools, ot_b, og,
                )

            with (
                tc.tile_pool(name="qkv_acc", bufs=1) as accp,
                tc.tile_pool(name="qkv_xt", bufs=2) as xtp,
                tc.tile_pool(name="qkv_w", bufs=3) as wp,
            ):
                for qq in range(NQ):
                    q_tiles[qq] = qp.tile(
                        [128, HPC, SEQ_BLK], BF16, name=f"q_sb{qq}", tag="q"
                    )
                    qkv = _qkv_quarter_gen(
                        nc, qq, xt, wqt, wkt, wvt, q_tiles[qq], k_q[qq], v_q[qq],
                        accp, xtp, wp, psA, psB, psD,
                    )
                    # pace two attention tiles of chunk qq-1 after every
                    # QKV PSUM group: each tile's exp hides under ~2.1us of
                    # QKV matmul work that doesn't depend on it
                    if qq == 0:
                        _interleave(qkv, None, 1)
                    else:
                        _interleave(qkv, attn_gen(qq - 1), 1, per=2)
                    if qq == 0:
                        for t in range(4):
                            nc.scalar.dma_start(mask_sb[:, t, :], dmask[t, :, :])
                        nc.scalar.dma_start(ones_sb[:, :], ones[:, :])
                    if qq == 1:
                        wcols = _load_wcols(nc, wcp, wot_t)
            with tc.tile_pool(name="op_og", bufs=6) as ogp:
                op_subs = []
                for ci, (q0, qlen) in enumerate(CHUNKS):
                    for off in range(0, qlen, OSUB):
                        op_subs.append((ci, q0, off))
                # chunk 3/4 attention paced over o_proj of chunks 0/1 so the
                # last AllGathers fire early; then drain o_proj 2..4
                _interleave(attn_gen(3),
                            _oproj_gen(nc, ogp, yp, psD, og, wcols, out, op_subs[0:2]),
                            8)
                _interleave(attn_gen(4),
                            _oproj_gen(nc, ogp, yp, psD, og, wcols, out, op_subs[2:4]),
                            9)
                _interleave(
                    _oproj_gen(nc, ogp, yp, psD, og, wcols, out, op_subs[4:8]),
                    None, 1)

    nc.compile()
    return nc


def _qkv_quarter_gen(nc, qq, xt, wqt, wkt, wvt, q_sb, k_sb, v_sb,
                     accp, xtp, wp, psA, psB, psD):
    s0 = qq * SEQ_BLK
    acc_q = accp.tile([128, HPC, SEQ_BLK], F32, name=f"accq{qq}", tag="accq")
    acc_k = accp.tile([128, HPC, SEQ_BLK], F32, name=f"acck{qq}", tag="acck")
    acc_v = accp.tile([128, SEQ_BLK // 128, DPC], F32, name=f"accv{qq}", tag="accv")
    for b in range(NSB):
        h0 = b * SB
        xtb = xtp.tile([128, NST, SEQ_BLK], BF16, name=f"xtb{qq}_{b}", tag="xtb")
        first = qq == 0 and b <= 2
        for i in range(NST):
            xeng = nc.scalar if (first and i % 2 == 1) else nc.sync
            xeng.dma_start(
                xtb[:, i, :], xt[h0 + 128 * i : h0 + 128 * (i + 1), s0 : s0 + SEQ_BLK]
            )
        for wsrc, acc, fin, weng, pname in (
            (wqt, acc_q, q_sb, nc.gpsimd, "q"),
            (wkt, acc_k, k_sb, nc.gpsimd if first else nc.sync, "k"),
        ):
            w = wp.tile([128, NST, DPC], BF16, name=f"w{pname}{qq}_{b}", tag="wrow")
            for i in range(NST):
                weng.dma_start(
                    w[:, i, :], wsrc[h0 + 128 * i : h0 + 128 * (i + 1), :]
                )
            for dt in range(HPC):
                # very first block: two half-contraction groups so the PE
                # can finish work as soon as half the (cold) DMAs land
                halves = ((0, 4), (4, 8)) if first else ((0, NST),)
                for gi, (i0, i1) in enumerate(halves):
                    pool = psA if dt % 2 == 0 else psD
                    tag = "ps" if dt % 2 == 0 else "py"
                    ps = pool.tile([128, 512], F32, name=f"ps{pname}", tag=tag)
                    for i in range(i0, i1):
                        nc.tensor.matmul(
                            ps[:, :],
                            lhsT=w[:, i, 128 * dt : 128 * (dt + 1)],
                            rhs=xtb[:, i, :],
                            start=(i == i0),
                            stop=(i == i1 - 1),
                        )
                    acc_sl = acc[:, dt, :]
                    if b == 0 and gi == 0:
                        nc.vector.tensor_copy(acc_sl, ps[:, :])
                    elif b < NSB - 1 or (first and gi == 1):
                        nc.vector.tensor_add(acc_sl, acc_sl, ps[:, :])
                    else:
                        # final add rounds straight into the bf16 store
                        nc.vector.tensor_add(fin[:, dt, :], acc_sl, ps[:, :])
                    yield
        # V pass: natural layout [seq, d]
        w = wp.tile([128, NST, DPC], BF16, name=f"wv{qq}_{b}", tag="wrow")
        for i in range(NST):
            nc.scalar.dma_start(
                w[:, i, :], wvt[h0 + 128 * i : h0 + 128 * (i + 1), :]
            )
        for st in range(SEQ_BLK // 128):
            for nh in range(2):
                g = 2 * st + nh
                pool = psB if g % 2 == 0 else psD
                tag = "po" if g % 2 == 0 else "py"
                ps = pool.tile([128, 512], F32, name="psv", tag=tag)
                for i in range(NST):
                    nc.tensor.matmul(
                        ps[:, :320],
                        lhsT=xtb[:, i, 128 * st : 128 * (st + 1)],
                        rhs=w[:, i, 320 * nh : 320 * (nh + 1)],
                        start=(i == 0),
                        stop=(i == NST - 1),
                    )
                acc_sl = acc_v[:, st, 320 * nh : 320 * (nh + 1)]
                if b == 0:
                    nc.vector.tensor_copy(acc_sl, ps[:, :320])
                elif b < NSB - 1:
                    nc.vector.tensor_add(acc_sl, acc_sl, ps[:, :320])
                else:
                    nc.vector.tensor_add(
                        v_sb[:, st, 320 * nh : 320 * (nh + 1)], acc_sl, ps[:, :320]
                    )
                yield


def _attn_tail(nc, pp, v_q, po, ptsum, ones_sb, mask_sb, ci, h, pending,
               pos_of, nk, kd0, qlen):
    ps, kt = pending
    pos = pos_of[kt]
    ptile = pp.tile([128, 512], BF16, name=f"pt{ci}_{h}_{kt}", tag="pt")
    nc.scalar.activation(
        ptile[:, :qlen], ps[:, :qlen], mybir.ActivationFunctionType.Exp,
        bias=0.0, scale=SCALE,
    )
    if kt >= kd0:
        # zero out future keys: multiply by the {0,1} diagonal mask
        ptm = pp.tile([128, 512], BF16, name=f"ptm{ci}_{h}_{kt}", tag="ptm", bufs=3)
        nc.vector.tensor_mul(
            ptm[:, :qlen], ptile[:, :qlen], mask_sb[:, kt - kd0, :qlen]
        )
        ptile = ptm
    nc.tensor.matmul(
        po[:, :qlen],
        lhsT=v_q[kt // 4][:, kt % 4, 128 * h : 128 * (h + 1)],
        rhs=ptile[:, :qlen],
        start=(pos == 0), stop=(pos == nk - 1),
    )
    # denominator accumulation on DVE (off the PE): ptsum += exp tile
    if pos == 0:
        nc.vector.tensor_copy(ptsum[:, :qlen], ptile[:, :qlen])
    else:
        nc.vector.tensor_add(ptsum[:, :qlen], ptsum[:, :qlen], ptile[:, :qlen])


def _attn_chunk_gen(nc, ci, q_sb, k_q, v_q, mask_sb, ones_sb, pools, ot_b, og):
    (psA, psB, psD, pp, op_) = pools
    q0, qlen = CHUNKS[ci]
    nk = (q0 + qlen) // 128
    kd0 = q0 // 128
    order = list(range(nk))
    pos_of = {kt: i for i, kt in enumerate(order)}
    for h in range(HPC):
        qtile = q_sb[:, h, (q0 % SEQ_BLK) : (q0 % SEQ_BLK) + qlen]
        po = psB.tile([128, 512], F32, name=f"po{ci}_{h}", tag="po")
        ptsum = pp.tile([128, 512], BF16, name=f"ptsum{ci}_{h}", tag="ptsum", bufs=2)

        # 2-deep software pipeline: emit S-matmuls two tiles ahead of
        # the exp/PV consumers so PE never waits on the exp chain.
        queue = []
        for kt in order:
            ps = psA.tile([128, 512], F32, name=f"ps{ci}_{h}_{kt}", tag="ps")
            nc.tensor.matmul(
                ps[:, :qlen],
                lhsT=k_q[kt // 4][:, h, 128 * (kt % 4) : 128 * (kt % 4 + 1)],
                rhs=qtile,
                start=True,
                stop=True,
            )
            queue.append((ps, kt))
            if len(queue) > 3:
                _attn_tail(nc, pp, v_q, po, ptsum, ones_sb, mask_sb, ci, h,
                           queue.pop(0), pos_of, nk, kd0, qlen)
            yield
        for pend in queue:
            _attn_tail(nc, pp, v_q, po, ptsum, ones_sb, mask_sb, ci, h,
                       pend, pos_of, nk, kd0, qlen)
            yield

        # single ones-matmul turns the DVE-accumulated exp sums into the
        # softmax denominators for all 512 queries of this head
        pl = psD.tile([128, 512], F32, name=f"pl{ci}_{h}", tag="py")
        nc.tensor.matmul(
            pl[:, :qlen], lhsT=ones_sb[:, :], rhs=ptsum[:, :qlen],
            start=True, stop=True,
        )
        linv = op_.tile([128, 512], F32, name=f"linv{ci}_{h}", tag="linv")
        _scalar_reciprocal(nc, linv[:, :qlen], pl[:, :qlen])
        ot = op_.tile([128, 512], BF16, name=f"ot{ci}_{h}", tag="ot", bufs=2)
        nc.vector.tensor_mul(ot[:, :qlen], po[:, :qlen], linv[:, :qlen])
        nc.gpsimd.dma_start(ot_b[ci][128 * h : 128 * (h + 1), :], ot[:, :qlen])
        yield
    nc.gpsimd.collective_compute(
        "AllGather",
        mybir.AluOpType.bypass,
        replica_groups=RG,
        ins=[ot_b[ci].ap().opt()],
        outs=[og[ci].ap().opt()],
    )


def _load_wcols(nc, wcp, wot_t):
    """o_proj weights, SBUF-resident for the whole tail of the kernel."""
    wcols = {}
    for mt in range(HPC):
        for piece in range(2):
            wcol = wcp.tile(
                [128, NH // 2, 128], BF16, name=f"wo{mt}_{piece}", tag=f"wc{mt}_{piece}"
            )
            nc.scalar.dma_start(
                wcol[:, :, :],
                wot_t[mt, :, piece * (HID // 2) : (piece + 1) * (HID // 2)].rearrange(
                    "p (a b) -> p a b", a=NH // 2
                ),
            )
            wcols[(mt, piece)] = wcol
    return wcols


def _oproj_gen(nc, ogp, yp, psD, og, wcols, out, subs):
    """o_proj: full 5120 contraction per (sub-chunk, out-tile) PSUM group,
    streamed in 256-col sub-chunks; yields once per PSUM group."""
    for ci, q0, off in subs:
        halves = []
        for piece in range(2):
            ogt = ogp.tile([128, NH // 2, OSUB], BF16,
                           name=f"ogt{ci}_{off}_{piece}", tag="ogt")
            for hh in range(NH // 2):
                ht = piece * (NH // 2) + hh
                nc.sync.dma_start(
                    ogt[:, hh, :], og[ci][128 * ht : 128 * (ht + 1), off : off + OSUB]
                )
            halves.append(ogt)
        for mt in range(HPC):
            ps = psD.tile([128, 512], F32, name=f"py{ci}_{off}_{mt}", tag="py")
            for ht in range(NH):
                piece, hh = divmod(ht, NH // 2)
                nc.tensor.matmul(
                    ps[:, :OSUB],
                    lhsT=wcols[(mt, piece)][:, hh, :],
                    rhs=halves[piece][:, hh, :],
                    start=(ht == 0),
                    stop=(ht == NH - 1),
                )
            ysb = yp.tile([128, 512], F32, name=f"y{ci}_{off}_{mt}", tag="y")
            nc.vector.tensor_copy(ysb[:, :OSUB], ps[:, :OSUB])
            nc.gpsimd.dma_start(
                out[128 * mt : 128 * (mt + 1), q0 + off : q0 + off + OSUB],
                ysb[:, :OSUB],
            )
            yield


def _to_bf16(a):
    return np.asarray(a, dtype=np.float32).astype(ml_dtypes.bfloat16)


def _prep_inputs(hidden_states, W_pack_w, o_proj_w):
    xt = _to_bf16(np.ascontiguousarray(hidden_states.reshape(S, HID).T))
    # dmask[t, p, q] = 1 where key (128t + p) may be attended by query q
    # of the diagonal region (q >= 128t + p), 0 otherwise
    dmask = np.zeros((4, 128, 512), dtype=np.float32)
    for t in range(4):
        for p in range(128):
            k = 128 * t + p
            dmask[t, p, min(k, 512):] = 1.0
    dmask = dmask.astype(ml_dtypes.bfloat16)
    ones = np.ones((128, 128), dtype=ml_dtypes.bfloat16)
    in_maps = []
    for c in range(N_CORES):
        r0 = DPC * c
        # wot_t[mt][p][ht*128+c2] = o_proj_w[r0 + 128 mt + c2, 128 ht + p]
        woc = o_proj_w[r0 : r0 + DPC, :]          # [640 out, 5120 in]
        wot_t = np.ascontiguousarray(
            woc.reshape(HPC, 128, NH, 128).transpose(0, 3, 2, 1).reshape(HPC, 128, HID)
        )
        in_maps.append(
            {
                "xt": xt,
                "wqt": _to_bf16(np.ascontiguousarray(W_pack_w[r0 : r0 + DPC, :].T)),
                "wkt": _to_bf16(np.ascontiguousarray(W_pack_w[HID + r0 : HID + r0 + DPC, :].T)),
                "wvt": _to_bf16(np.ascontiguousarray(W_pack_w[2 * HID + r0 : 2 * HID + r0 + DPC, :].T)),
                "wot_t": _to_bf16(wot_t),
                "dmask": dmask,
                "ones": ones,
            }
        )
    return in_maps


def run(hidden_states, W_pack_w, o_proj_w, trace=False):
    global _graph_cache
    if _graph_cache is None:
        _graph_cache = _build_graph()
    nc = _graph_cache
    in_maps = _prep_inputs(hidden_states, W_pack_w, o_proj_w)
    res = run_bass_kernel_spmd(nc, in_maps, list(range(N_CORES)), trace=trace)
    y = np.concatenate([res.results[c]["out"].T for c in range(N_CORES)], axis=1)
    return y.reshape(1, S, HID), res


def kernel(
    hidden_states,
    W_pack_w,
    o_proj_w,
    k_cache=None,
    v_cache=None,
    input_pos=None,
    attention_mask=None,
    **_unused,
):
    hidden_states = np.asarray(hidden_states, dtype=np.float32)
    W_pack_w = np.asarray(W_pack_w, dtype=np.float32)
    o_proj_w = np.asarray(o_proj_w, dtype=np.float32)
    y, _ = run(hidden_states, W_pack_w, o_proj_w, trace=False)
    return y


# revision 30
# speedup vs baseline: 1.0008x; 1.0008x over previous
"""Baichuan attention prefill on 8 TRN2 NeuronCores.

Tensor-parallel over heads: 5 heads per core. Per core:
  QKV projection (its heads' slice of W_pack) -> attention (causal,
  softmax without max-subtraction) -> AllGather of attention output
  O^T across cores -> o_proj with full contraction producing this
  core's 640 output dims. Host assembles the [1, 2048, 5120] output.

v6: fine-grained interleaving. Attention chunk ci's tiles are emitted
interspersed with QKV quarter ci+1's PSUM groups (and o_proj groups
under the last chunks), so the scalar-engine exp latency - the real
attention ceiling - is hidden under PE matmul work that does not
depend on it. Softmax denominators come from a DVE-accumulated sum of
the exp tiles followed by a single ones-matmul per (chunk, head)
instead of a ones-matmul per key tile, cutting the PE L-pass ~10x.
Reciprocal runs on the scalar engine. One set of PSUM pools with
shared tags across all phases keeps the PE queue barrier-free.
"""

import numpy as np
import ml_dtypes

import concourse.bacc as bacc
import concourse.mybir as mybir
from concourse.tile import TileContext
from concourse.bass_utils import run_bass_kernel_spmd

HID = 5120
NH = 40
HD = 128
S = 2048
N_CORES = 8
HPC = NH // N_CORES          # 5 heads per core
DPC = HPC * HD               # 640 dims per core
F32 = mybir.dt.float32
BF16 = mybir.dt.bfloat16
SCALE = 1.0 / float(np.sqrt(HD))

SB = 1024                    # hid superblock (8 x 128 subtiles)
NSB = HID // SB              # 5 superblocks
NST = SB // 128              # 8 subtiles per superblock
SEQ_BLK = 512                # QKV seq quarter
NQ = S // SEQ_BLK            # 4
# attention / AllGather chunks (q0, qlen); finer at the end so the last
# AllGather lands early and o_proj can finish right behind it
CHUNKS = [(0, 512), (512, 512), (1024, 512), (1536, 256), (1792, 256)]
OSUB = 256                   # o_proj column sub-chunk
RG = [list(range(N_CORES))]

_graph_cache = None


def _scalar_reciprocal(nc, out, in_):
    """Reciprocal on the Activation engine (~0.7us vs ~4us on DVE for a
    [128,512] tile). The softmax denominator tolerates activation-table
    accuracy (verified against the fp64 oracle)."""
    eng = nc.scalar
    ins = [eng.lower_ap(in_)]
    for v in (0.0, 1.0, 0.0):
        ins.append(mybir.ImmediateValue(dtype=mybir.dt.float32, value=v))
    return eng.add_instruction(
        mybir.InstActivation(
            name=eng.bass.get_next_instruction_name(),
            func=mybir.ActivationFunctionType.Reciprocal,
            ins=ins,
            outs=[eng.lower_ap(out)],
        )
    )


def _interleave(primary, inserts, ratio, per=1, lead=0):
    """Drive two emission generators: advance `primary` and insert `per`
    steps of `inserts` every `ratio` primary steps. `lead` advances the
    insert stream first (prefetches its DMAs). Drains both."""
    for _ in range(lead):
        next(inserts, None)
    i = 0
    for _ in primary:
        i += 1
        if inserts is not None and i % ratio == 0:
            for _ in range(per):
                next(inserts, None)
    if inserts is not None:
        for _ in inserts:
            pass


def _build_graph():
    nc = bacc.Bacc(name="baichuan_attn")

    xt = nc.declare_dram_parameter("xt", [HID, S], BF16, isOutput=False)
    wqt = nc.declare_dram_parameter("wqt", [HID, DPC], BF16, isOutput=False)
    wkt = nc.declare_dram_parameter("wkt", [HID, DPC], BF16, isOutput=False)
    wvt = nc.declare_dram_parameter("wvt", [HID, DPC], BF16, isOutput=False)
    # wot_t[mt][p][ht*128+c] = o_proj_w[640c_core + 128 mt + c, 128 ht + p]
    wot_t = nc.declare_dram_parameter("wot_t", [HPC, 128, HID], BF16, isOutput=False)
    # {0,1} causal masks for diagonal key tiles, by relative tile offset
    dmask = nc.declare_dram_parameter("dmask", [4, 128, 512], BF16, isOutput=False)
    ones = nc.declare_dram_parameter("ones", [128, 128], BF16, isOutput=False)
    out = nc.declare_dram_parameter("out", [DPC, S], F32, isOutput=True)

    warm_in = nc.dram_tensor("warm_in", [128, 8], BF16)
    warm_out = nc.dram_tensor("warm_out", [1024, 8], BF16, addr_space="Shared")
    ot_b = [
        nc.dram_tensor(f"ot_b{ci}", [DPC, qlen], BF16)
        for ci, (q0, qlen) in enumerate(CHUNKS)
    ]
    og = [
        nc.dram_tensor(f"og{ci}", [HID, qlen], BF16, addr_space="Shared")
        for ci, (q0, qlen) in enumerate(CHUNKS)
    ]

    with TileContext(nc) as tc:
        nc.gpsimd.collective_compute(
            "AllGather",
            mybir.AluOpType.bypass,
            replica_groups=RG,
            ins=[warm_in.ap().opt()],
            outs=[warm_out.ap().opt()],
        )
        with (
            # PSUM pools shared by every phase (same tags) so no pool
            # boundary ever blocks the PE queue: 4+2+2 = 8 banks.
            tc.tile_pool(name="psA", bufs=4, space="PSUM") as psA,
            tc.tile_pool(name="psB", bufs=2, space="PSUM") as psB,
            tc.tile_pool(name="psD", bufs=2, space="PSUM") as psD,
            tc.tile_pool(name="kv_sb", bufs=1) as kvp,
            tc.tile_pool(name="q_sb", bufs=2) as qp,
            tc.tile_pool(name="cst", bufs=1) as cstp,
            tc.tile_pool(name="at_p", bufs=6) as pp,
            tc.tile_pool(name="at_o", bufs=1) as op_,
            tc.tile_pool(name="op_y", bufs=3) as yp,
            tc.tile_pool(name="op_w", bufs=1) as wcp,
        ):
            k_q = [
                kvp.tile([128, HPC, SEQ_BLK], BF16, name=f"k_sb{qq}")
                for qq in range(NQ)
            ]
            v_q = [
                kvp.tile([128, SEQ_BLK // 128, DPC], BF16, name=f"v_sb{qq}")
                for qq in range(NQ)
            ]
            mask_sb = cstp.tile([128, 4, 512], BF16, name="mask_sb")
            ones_sb = cstp.tile([128, 128], BF16, name="ones_sb")

            pools = (psA, psB, psD, pp, op_)
            q_tiles = {}

            def attn_gen(ci):
                return _attn_chunk_gen(
                    nc, ci, q_tiles[CHUNKS[ci][0] // SEQ_BLK], k_q, v_q,
                    mask_sb, ones_sb, pools, ot_b, og,
                )

            with (
                tc.tile_pool(name="qkv_acc", bufs=1) as accp,
                tc.tile_pool(name="qkv_xt", bufs=2) as xtp,
                tc.tile_pool(name="qkv_w", bufs=3) as wp,
            ):
                for qq in range(NQ):
                    q_tiles[qq] = qp.tile(
                        [128, HPC, SEQ_BLK], BF16, name=f"q_sb{qq}", tag="q"
                    )
                    qkv = _qkv_quarter_gen(
                        nc, qq, xt, wqt, wkt, wvt, q_tiles[qq], k_q[qq], v_q[qq],
                        accp, xtp, wp, psA, psB, psD,
                    )
                    # pace two attention tiles of chunk qq-1 after every
                    # QKV PSUM group: each tile's exp hides under ~2.1us of
                    # QKV matmul work that doesn't depend on it
                    if qq == 0:
                        _interleave(qkv, None, 1)
                    else:
                        _interleave(qkv, attn_gen(qq - 1), 1, per=2)
                    if qq == 0:
                        for t in range(4):
                            nc.scalar.dma_start(mask_sb[:, t, :], dmask[t, :, :])
                        nc.scalar.dma_start(ones_sb[:, :], ones[:, :])
                    if qq == 1:
                        wcols = _load_wcols(nc, wcp, wot_t)
            with tc.tile_pool(name="op_og", bufs=6) as ogp:
                op_subs = []
                for ci, (q0, qlen) in enumerate(CHUNKS):
                    for off in range(0, qlen, OSUB):
                        op_subs.append((ci, q0, off))
                # chunk 3/4 attention paced over o_proj of chunks 0/1 so the
                # last AllGathers fire early; then drain o_proj 2..4
                _interleave(attn_gen(3),
                            _oproj_gen(nc, ogp, yp, psD, og, wcols, out, op_subs[0:2]),
                            8, lead=1)
                _interleave(attn_gen(4),
                            _oproj_gen(nc, ogp, yp, psD, og, wcols, out, op_subs[2:4]),
                            9)
                _interleave(
                    _oproj_gen(nc, ogp, yp, psD, og, wcols, out, op_subs[4:8]),
                    None, 1)

    nc.compile()
    return nc


def _qkv_quarter_gen(nc, qq, xt, wqt, wkt, wvt, q_sb, k_sb, v_sb,
                     accp, xtp, wp, psA, psB, psD):
    s0 = qq * SEQ_BLK
    acc_q = accp.tile([128, HPC, SEQ_BLK], F32, name=f"accq{qq}", tag="accq")
    acc_k = accp.tile([128, HPC, SEQ_BLK], F32, name=f"acck{qq}", tag="acck")
    acc_v = accp.tile([128, SEQ_BLK // 128, DPC], F32, name=f"accv{qq}", tag="accv")
    for b in range(NSB):
        h0 = b * SB
        xtb = xtp.tile([128, NST, SEQ_BLK], BF16, name=f"xtb{qq}_{b}", tag="xtb")
        first = qq == 0 and b <= 1
        for i in range(NST):
            xeng = nc.scalar if (first and i % 2 == 1) else nc.sync
            xeng.dma_start(
                xtb[:, i, :], xt[h0 + 128 * i : h0 + 128 * (i + 1), s0 : s0 + SEQ_BLK]
            )
        for wsrc, acc, fin, weng, pname in (
            (wqt, acc_q, q_sb, nc.gpsimd, "q"),
            (wkt, acc_k, k_sb, nc.gpsimd if first else nc.sync, "k"),
        ):
            w = wp.tile([128, NST, DPC], BF16, name=f"w{pname}{qq}_{b}", tag="wrow")
            for i in range(NST):
                weng.dma_start(
                    w[:, i, :], wsrc[h0 + 128 * i : h0 + 128 * (i + 1), :]
                )
            for dt in range(HPC):
                # very first block: two half-contraction groups so the PE
                # can finish work as soon as half the (cold) DMAs land
                halves = ((0, 4), (4, 8)) if first else ((0, NST),)
                for gi, (i0, i1) in enumerate(halves):
                    pool = psA if dt % 2 == 0 else psD
                    tag = "ps" if dt % 2 == 0 else "py"
                    ps = pool.tile([128, 512], F32, name=f"ps{pname}", tag=tag)
                    for i in range(i0, i1):
                        nc.tensor.matmul(
                            ps[:, :],
                            lhsT=w[:, i, 128 * dt : 128 * (dt + 1)],
                            rhs=xtb[:, i, :],
                            start=(i == i0),
                            stop=(i == i1 - 1),
                        )
                    acc_sl = acc[:, dt, :]
                    if b == 0 and gi == 0:
                        nc.vector.tensor_copy(acc_sl, ps[:, :])
                    elif b < NSB - 1 or (first and gi == 1):
                        nc.vector.tensor_add(acc_sl, acc_sl, ps[:, :])
                    else:
                        # final add rounds straight into the bf16 store
                        nc.vector.tensor_add(fin[:, dt, :], acc_sl, ps[:, :])
                    yield
        # V pass: natural layout [seq, d]
        w = wp.tile([128, NST, DPC], BF16, name=f"wv{qq}_{b}", tag="wrow")
        for i in range(NST):
            nc.scalar.dma_start(
                w[:, i, :], wvt[h0 + 128 * i : h0 + 128 * (i + 1), :]
            )
        for st in range(SEQ_BLK // 128):
            for nh in range(2):
                g = 2 * st + nh
                pool = psB if g % 2 == 0 else psD
                tag = "po" if g % 2 == 0 else "py"
                ps = pool.tile([128, 512], F32, name="psv", tag=tag)
                for i in range(NST):
                    nc.tensor.matmul(
                        ps[:, :320],
                        lhsT=xtb[:, i, 128 * st : 128 * (st + 1)],
                        rhs=w[:, i, 320 * nh : 320 * (nh + 1)],
                        start=(i == 0),
                        stop=(i == NST - 1),
                    )
                acc_sl = acc_v[:, st, 320 * nh : 320 * (nh + 1)]
                if b == 0:
                    nc.vector.tensor_copy(acc_sl, ps[:, :320])
                elif b < NSB - 1:
                    nc.vector.tensor_add(acc_sl, acc_sl, ps[:, :320])
                else:
                    nc.vector.tensor_add(
                        v_sb[:, st, 320 * nh : 320 * (nh + 1)], acc_sl, ps[:, :320]
                    )
                yield


def _attn_tail(nc, pp, v_q, po, ptsum, ones_sb, mask_sb, ci, h, pending,
               pos_of, nk, kd0, qlen):
    ps, kt = pending
    pos = pos_of[kt]
    ptile = pp.tile([128, 512], BF16, name=f"pt{ci}_{h}_{kt}", tag="pt")
    nc.scalar.activation(
        ptile[:, :qlen], ps[:, :qlen], mybir.ActivationFunctionType.Exp,
        bias=0.0, scale=SCALE,
    )
    if kt >= kd0:
        # zero out future keys: multiply by the {0,1} diagonal mask
        ptm = pp.tile([128, 512], BF16, name=f"ptm{ci}_{h}_{kt}", tag="ptm", bufs=3)
        nc.vector.tensor_mul(
            ptm[:, :qlen], ptile[:, :qlen], mask_sb[:, kt - kd0, :qlen]
        )
        ptile = ptm
    nc.tensor.matmul(
        po[:, :qlen],
        lhsT=v_q[kt // 4][:, kt % 4, 128 * h : 128 * (h + 1)],
        rhs=ptile[:, :qlen],
        start=(pos == 0), stop=(pos == nk - 1),
    )
    # denominator accumulation on DVE (off the PE): ptsum += exp tile
    if pos == 0:
        nc.vector.tensor_copy(ptsum[:, :qlen], ptile[:, :qlen])
    else:
        nc.vector.tensor_add(ptsum[:, :qlen], ptsum[:, :qlen], ptile[:, :qlen])


def _attn_chunk_gen(nc, ci, q_sb, k_q, v_q, mask_sb, ones_sb, pools, ot_b, og):
    (psA, psB, psD, pp, op_) = pools
    q0, qlen = CHUNKS[ci]
    nk = (q0 + qlen) // 128
    kd0 = q0 // 128
    order = list(range(nk))
    pos_of = {kt: i for i, kt in enumerate(order)}
    for h in range(HPC):
        qtile = q_sb[:, h, (q0 % SEQ_BLK) : (q0 % SEQ_BLK) + qlen]
        po = psB.tile([128, 512], F32, name=f"po{ci}_{h}", tag="po")
        ptsum = pp.tile([128, 512], BF16, name=f"ptsum{ci}_{h}", tag="ptsum", bufs=2)

        # 2-deep software pipeline: emit S-matmuls two tiles ahead of
        # the exp/PV consumers so PE never waits on the exp chain.
        queue = []
        for kt in order:
            ps = psA.tile([128, 512], F32, name=f"ps{ci}_{h}_{kt}", tag="ps")
            nc.tensor.matmul(
                ps[:, :qlen],
                lhsT=k_q[kt // 4][:, h, 128 * (kt % 4) : 128 * (kt % 4 + 1)],
                rhs=qtile,
                start=True,
                stop=True,
            )
            queue.append((ps, kt))
            if len(queue) > 3:
                _attn_tail(nc, pp, v_q, po, ptsum, ones_sb, mask_sb, ci, h,
                           queue.pop(0), pos_of, nk, kd0, qlen)
            yield
        for pend in queue:
            _attn_tail(nc, pp, v_q, po, ptsum, ones_sb, mask_sb, ci, h,
                       pend, pos_of, nk, kd0, qlen)
            yield

        # single ones-matmul turns the DVE-accumulated exp sums into the
        # softmax denominators for all 512 queries of this head
        pl = psD.tile([128, 512], F32, name=f"pl{ci}_{h}", tag="py")
        nc.tensor.matmul(
            pl[:, :qlen], lhsT=ones_sb[:, :], rhs=ptsum[:, :qlen],
            start=True, stop=True,
        )
        linv = op_.tile([128, 512], F32, name=f"linv{ci}_{h}", tag="linv")
        _scalar_reciprocal(nc, linv[:, :qlen], pl[:, :qlen])
        ot = op_.tile([128, 512], BF16, name=f"ot{ci}_{h}", tag="ot", bufs=2)
        nc.vector.tensor_mul(ot[:, :qlen], po[:, :qlen], linv[:, :qlen])
        nc.gpsimd.dma_start(ot_b[ci][128 * h : 128 * (h + 1), :], ot[:, :qlen])
        yield
    nc.gpsimd.collective_compute(
        "AllGather",
        mybir.AluOpType.bypass,
        replica_groups=RG,
        ins=[ot_b[ci].ap().opt()],
        outs=[og[ci].ap().opt()],
    )


def _load_wcols(nc, wcp, wot_t):
    """o_proj weights, SBUF-resident for the whole tail of the kernel."""
    wcols = {}
    for mt in range(HPC):
        for piece in range(2):
            wcol = wcp.tile(
                [128, NH // 2, 128], BF16, name=f"wo{mt}_{piece}", tag=f"wc{mt}_{piece}"
            )
            nc.scalar.dma_start(
                wcol[:, :, :],
                wot_t[mt, :, piece * (HID // 2) : (piece + 1) * (HID // 2)].rearrange(
                    "p (a b) -> p a b", a=NH // 2
                ),
            )
            wcols[(mt, piece)] = wcol
    return wcols


def _oproj_gen(nc, ogp, yp, psD, og, wcols, out, subs):
    """o_proj: full 5120 contraction per (sub-chunk, out-tile) PSUM group,
    streamed in 256-col sub-chunks; yields once per PSUM group."""
    for ci, q0, off in subs:
        halves = []
        for piece in range(2):
            ogt = ogp.tile([128, NH // 2, OSUB], BF16,
                           name=f"ogt{ci}_{off}_{piece}", tag="ogt")
            for hh in range(NH // 2):
                ht = piece * (NH // 2) + hh
                nc.sync.dma_start(
                    ogt[:, hh, :], og[ci][128 * ht : 128 * (ht + 1), off : off + OSUB]
                )
            halves.append(ogt)
        for mt in range(HPC):
            ps = psD.tile([128, 512], F32, name=f"py{ci}_{off}_{mt}", tag="py")
            for ht in range(NH):
                piece, hh = divmod(ht, NH // 2)
                nc.tensor.matmul(
                    ps[:, :OSUB],
                    lhsT=wcols[(mt, piece)][:, hh, :],
                    rhs=halves[piece][:, hh, :],
                    start=(ht == 0),
                    stop=(ht == NH - 1),
                )
            ysb = yp.tile([128, 512], F32, name=f"y{ci}_{off}_{mt}", tag="y")
            nc.vector.tensor_copy(ysb[:, :OSUB], ps[:, :OSUB])
            nc.gpsimd.dma_start(
                out[128 * mt : 128 * (mt + 1), q0 + off : q0 + off + OSUB],
                ysb[:, :OSUB],
            )
            yield


def _to_bf16(a):
    return np.asarray(a, dtype=np.float32).astype(ml_dtypes.bfloat16)


def _prep_inputs(hidden_states, W_pack_w, o_proj_w):
    xt = _to_bf16(np.ascontiguousarray(hidden_states.reshape(S, HID).T))
    # dmask[t, p, q] = 1 where key (128t + p) may be attended by query q
    # of the diagonal region (q >= 128t + p), 0 otherwise
    dmask = np.zeros((4, 128, 512), dtype=np.float32)
    for t in range(4):
        for p in range(128):
            k = 128 * t + p
            dmask[t, p, min(k, 512):] = 1.0
    dmask = dmask.astype(ml_dtypes.bfloat16)
    ones = np.ones((128, 128), dtype=ml_dtypes.bfloat16)
    in_maps = []
    for c in range(N_CORES):
        r0 = DPC * c
        # wot_t[mt][p][ht*128+c2] = o_proj_w[r0 + 128 mt + c2, 128 ht + p]
        woc = o_proj_w[r0 : r0 + DPC, :]          # [640 out, 5120 in]
        wot_t = np.ascontiguousarray(
            woc.reshape(HPC, 128, NH, 128).transpose(0, 3, 2, 1).reshape(HPC, 128, HID)
        )
        in_maps.append(
            {
                "xt": xt,
                "wqt": _to_bf16(np.ascontiguousarray(W_pack_w[r0 : r0 + DPC, :].T)),
                "wkt": _to_bf16(np.ascontiguousarray(W_pack_w[HID + r0 : HID + r0 + DPC, :].T)),
                "wvt": _to_bf16(np.ascontiguousarray(W_pack_w[2 * HID + r0 : 2 * HID + r0 + DPC, :].T)),
                "wot_t": _to_bf16(wot_t),
                "dmask": dmask,
                "ones": ones,
            }
        )
    return in_maps


def run(hidden_states, W_pack_w, o_proj_w, trace=False):
    global _graph_cache
    if _graph_cache is None:
        _graph_cache = _build_graph()
    nc = _graph_cache
    in_maps = _prep_inputs(hidden_states, W_pack_w, o_proj_w)
    res = run_bass_kernel_spmd(nc, in_maps, list(range(N_CORES)), trace=trace)
    y = np.concatenate([res.results[c]["out"].T for c in range(N_CORES)], axis=1)
    return y.reshape(1, S, HID), res


def kernel(
    hidden_states,
    W_pack_w,
    o_proj_w,
    k_cache=None,
    v_cache=None,
    input_pos=None,
    attention_mask=None,
    **_unused,
):
    hidden_states = np.asarray(hidden_states, dtype=np.float32)
    W_pack_w = np.asarray(W_pack_w, dtype=np.float32)
    o_proj_w = np.asarray(o_proj_w, dtype=np.float32)
    y, _ = run(hidden_states, W_pack_w, o_proj_w, trace=False)
    return y


# revision 31
# speedup vs baseline: 1.0028x; 1.0019x over previous
"""Baichuan attention prefill on 8 TRN2 NeuronCores.

Tensor-parallel over heads: 5 heads per core. Per core:
  QKV projection (its heads' slice of W_pack) -> attention (causal,
  softmax without max-subtraction) -> AllGather of attention output
  O^T across cores -> o_proj with full contraction producing this
  core's 640 output dims. Host assembles the [1, 2048, 5120] output.

v6: fine-grained interleaving. Attention chunk ci's tiles are emitted
interspersed with QKV quarter ci+1's PSUM groups (and o_proj groups
under the last chunks), so the scalar-engine exp latency - the real
attention ceiling - is hidden under PE matmul work that does not
depend on it. Softmax denominators come from a DVE-accumulated sum of
the exp tiles followed by a single ones-matmul per (chunk, head)
instead of a ones-matmul per key tile, cutting the PE L-pass ~10x.
Reciprocal runs on the scalar engine. One set of PSUM pools with
shared tags across all phases keeps the PE queue barrier-free.
"""

import numpy as np
import ml_dtypes

import concourse.bacc as bacc
import concourse.mybir as mybir
from concourse.tile import TileContext
from concourse.bass_utils import run_bass_kernel_spmd

HID = 5120
NH = 40
HD = 128
S = 2048
N_CORES = 8
HPC = NH // N_CORES          # 5 heads per core
DPC = HPC * HD               # 640 dims per core
F32 = mybir.dt.float32
BF16 = mybir.dt.bfloat16
SCALE = 1.0 / float(np.sqrt(HD))

SB = 1024                    # hid superblock (8 x 128 subtiles)
NSB = HID // SB              # 5 superblocks
NST = SB // 128              # 8 subtiles per superblock
SEQ_BLK = 512                # QKV seq quarter
NQ = S // SEQ_BLK            # 4
# attention / AllGather chunks (q0, qlen); finer at the end so the last
# AllGather lands early and o_proj can finish right behind it
CHUNKS = [(0, 512), (512, 512), (1024, 512), (1536, 256), (1792, 256)]
OSUB = 256                   # o_proj column sub-chunk
RG = [list(range(N_CORES))]

_graph_cache = None


def _scalar_reciprocal(nc, out, in_):
    """Reciprocal on the Activation engine (~0.7us vs ~4us on DVE for a
    [128,512] tile). The softmax denominator tolerates activation-table
    accuracy (verified against the fp64 oracle)."""
    eng = nc.scalar
    ins = [eng.lower_ap(in_)]
    for v in (0.0, 1.0, 0.0):
        ins.append(mybir.ImmediateValue(dtype=mybir.dt.float32, value=v))
    return eng.add_instruction(
        mybir.InstActivation(
            name=eng.bass.get_next_instruction_name(),
            func=mybir.ActivationFunctionType.Reciprocal,
            ins=ins,
            outs=[eng.lower_ap(out)],
        )
    )


def _interleave(primary, inserts, ratio, per=1):
    """Drive two emission generators: advance `primary` and insert `per`
    steps of `inserts` every `ratio` primary steps. Drains both."""
    i = 0
    for _ in primary:
        i += 1
        if inserts is not None and i % ratio == 0:
            for _ in range(per):
                next(inserts, None)
    if inserts is not None:
        for _ in inserts:
            pass


def _build_graph():
    nc = bacc.Bacc(name="baichuan_attn")

    xt = nc.declare_dram_parameter("xt", [HID, S], BF16, isOutput=False)
    wqt = nc.declare_dram_parameter("wqt", [HID, DPC], BF16, isOutput=False)
    wkt = nc.declare_dram_parameter("wkt", [HID, DPC], BF16, isOutput=False)
    wvt = nc.declare_dram_parameter("wvt", [HID, DPC], BF16, isOutput=False)
    # wot_t[mt][p][ht*128+c] = o_proj_w[640c_core + 128 mt + c, 128 ht + p]
    wot_t = nc.declare_dram_parameter("wot_t", [HPC, 128, HID], BF16, isOutput=False)
    # {0,1} causal masks for diagonal key tiles, by relative tile offset
    dmask = nc.declare_dram_parameter("dmask", [4, 128, 512], BF16, isOutput=False)
    ones = nc.declare_dram_parameter("ones", [128, 128], BF16, isOutput=False)
    out = nc.declare_dram_parameter("out", [DPC, S], F32, isOutput=True)

    warm_in = nc.dram_tensor("warm_in", [128, 8], BF16)
    warm_out = nc.dram_tensor("warm_out", [1024, 8], BF16, addr_space="Shared")
    ot_b = [
        nc.dram_tensor(f"ot_b{ci}", [DPC, qlen], BF16)
        for ci, (q0, qlen) in enumerate(CHUNKS)
    ]
    og = [
        nc.dram_tensor(f"og{ci}", [HID, qlen], BF16, addr_space="Shared")
        for ci, (q0, qlen) in enumerate(CHUNKS)
    ]

    with TileContext(nc) as tc:
        nc.gpsimd.collective_compute(
            "AllGather",
            mybir.AluOpType.bypass,
            replica_groups=RG,
            ins=[warm_in.ap().opt()],
            outs=[warm_out.ap().opt()],
        )
        with (
            # PSUM pools shared by every phase (same tags) so no pool
            # boundary ever blocks the PE queue: 4+2+2 = 8 banks.
            tc.tile_pool(name="psA", bufs=4, space="PSUM") as psA,
            tc.tile_pool(name="psB", bufs=2, space="PSUM") as psB,
            tc.tile_pool(name="psD", bufs=2, space="PSUM") as psD,
            tc.tile_pool(name="kv_sb", bufs=1) as kvp,
            tc.tile_pool(name="q_sb", bufs=2) as qp,
            tc.tile_pool(name="cst", bufs=1) as cstp,
            tc.tile_pool(name="at_p", bufs=6) as pp,
            tc.tile_pool(name="at_o", bufs=1) as op_,
            tc.tile_pool(name="op_y", bufs=3) as yp,
            tc.tile_pool(name="op_w", bufs=1) as wcp,
        ):
            k_q = [
                kvp.tile([128, HPC, SEQ_BLK], BF16, name=f"k_sb{qq}")
                for qq in range(NQ)
            ]
            v_q = [
                kvp.tile([128, SEQ_BLK // 128, DPC], BF16, name=f"v_sb{qq}")
                for qq in range(NQ)
            ]
            mask_sb = cstp.tile([128, 4, 512], BF16, name="mask_sb")
            ones_sb = cstp.tile([128, 128], BF16, name="ones_sb")

            pools = (psA, psB, psD, pp, op_)
            q_tiles = {}

            def attn_gen(ci):
                return _attn_chunk_gen(
                    nc, ci, q_tiles[CHUNKS[ci][0] // SEQ_BLK], k_q, v_q,
                    mask_sb, ones_sb, pools, ot_b, og,
                )

            with (
                tc.tile_pool(name="qkv_acc", bufs=1) as accp,
                tc.tile_pool(name="qkv_xt", bufs=2) as xtp,
                tc.tile_pool(name="qkv_w", bufs=3) as wp,
            ):
                for qq in range(NQ):
                    q_tiles[qq] = qp.tile(
                        [128, HPC, SEQ_BLK], BF16, name=f"q_sb{qq}", tag="q"
                    )
                    qkv = _qkv_quarter_gen(
                        nc, qq, xt, wqt, wkt, wvt, q_tiles[qq], k_q[qq], v_q[qq],
                        accp, xtp, wp, psA, psB, psD,
                    )
                    # pace two attention tiles of chunk qq-1 after every
                    # QKV PSUM group: each tile's exp hides under ~2.1us of
                    # QKV matmul work that doesn't depend on it
                    if qq == 0:
                        _interleave(qkv, None, 1)
                    else:
                        _interleave(qkv, attn_gen(qq - 1), 1, per=2)
                    if qq == 0:
                        for t in range(4):
                            nc.scalar.dma_start(mask_sb[:, t, :], dmask[t, :, :])
                        nc.scalar.dma_start(ones_sb[:, :], ones[:, :])
                    if qq == 1:
                        wcols = _load_wcols(nc, wcp, wot_t)
            with tc.tile_pool(name="op_og", bufs=6) as ogp:
                op_subs = []
                for ci, (q0, qlen) in enumerate(CHUNKS):
                    for off in range(0, qlen, OSUB):
                        op_subs.append((ci, q0, off))
                # chunk 3/4 attention paced over o_proj of chunks 0/1 so the
                # last AllGathers fire early; then drain o_proj 2..4
                _interleave(attn_gen(3),
                            _oproj_gen(nc, ogp, yp, psD, og, wcols, out, op_subs[0:2]),
                            8)
                _interleave(attn_gen(4),
                            _oproj_gen(nc, ogp, yp, psD, og, wcols, out, op_subs[2:4]),
                            9)
                _interleave(
                    _oproj_gen(nc, ogp, yp, psD, og, wcols, out, op_subs[4:8]),
                    None, 1)

    nc.compile()
    return nc


def _qkv_quarter_gen(nc, qq, xt, wqt, wkt, wvt, q_sb, k_sb, v_sb,
                     accp, xtp, wp, psA, psB, psD):
    s0 = qq * SEQ_BLK
    acc_q = accp.tile([128, HPC, SEQ_BLK], F32, name=f"accq{qq}", tag="accq")
    acc_k = accp.tile([128, HPC, SEQ_BLK], F32, name=f"acck{qq}", tag="acck")
    acc_v = accp.tile([128, SEQ_BLK // 128, DPC], F32, name=f"accv{qq}", tag="accv")
    for b in range(NSB):
        h0 = b * SB
        xtb = xtp.tile([128, NST, SEQ_BLK], BF16, name=f"xtb{qq}_{b}", tag="xtb")
        first = qq == 0 and b <= 1
        for i in range(NST):
            xeng = nc.scalar if (first and i % 2 == 1) else nc.sync
            xeng.dma_start(
                xtb[:, i, :], xt[h0 + 128 * i : h0 + 128 * (i + 1), s0 : s0 + SEQ_BLK]
            )
        for wsrc, acc, fin, weng, pname in (
            (wqt, acc_q, q_sb, nc.gpsimd, "q"),
            (wkt, acc_k, k_sb, nc.gpsimd if first else nc.sync, "k"),
        ):
            w = wp.tile([128, NST, DPC], BF16, name=f"w{pname}{qq}_{b}", tag="wrow")
            for i in range(NST):
                weng.dma_start(
                    w[:, i, :], wsrc[h0 + 128 * i : h0 + 128 * (i + 1), :]
                )
            for dt in range(HPC):
                # very first block: two half-contraction groups so the PE
                # can finish work as soon as half the (cold) DMAs land
                halves = ((0, 4), (4, 8)) if first else ((0, NST),)
                for gi, (i0, i1) in enumerate(halves):
                    pool = psA if dt % 2 == 0 else psD
                    tag = "ps" if dt % 2 == 0 else "py"
                    ps = pool.tile([128, 512], F32, name=f"ps{pname}", tag=tag)
                    for i in range(i0, i1):
                        nc.tensor.matmul(
                            ps[:, :],
                            lhsT=w[:, i, 128 * dt : 128 * (dt + 1)],
                            rhs=xtb[:, i, :],
                            start=(i == i0),
                            stop=(i == i1 - 1),
                        )
                    acc_sl = acc[:, dt, :]
                    if b == 0 and gi == 0:
                        nc.vector.tensor_copy(acc_sl, ps[:, :])
                    elif b < NSB - 1 or (first and gi == 1):
                        nc.vector.tensor_add(acc_sl, acc_sl, ps[:, :])
                    else:
                        # final add rounds straight into the bf16 store
                        nc.vector.tensor_add(fin[:, dt, :], acc_sl, ps[:, :])
                    yield
        # V pass: natural layout [seq, d]
        w = wp.tile([128, NST, DPC], BF16, name=f"wv{qq}_{b}", tag="wrow")
        for i in range(NST):
            nc.scalar.dma_start(
                w[:, i, :], wvt[h0 + 128 * i : h0 + 128 * (i + 1), :]
            )
        for st in range(SEQ_BLK // 128):
            for nh in range(2):
                g = 2 * st + nh
                pool = psB if g % 2 == 0 else psD
                tag = "po" if g % 2 == 0 else "py"
                ps = pool.tile([128, 512], F32, name="psv", tag=tag)
                for i in range(NST):
                    nc.tensor.matmul(
                        ps[:, :320],
                        lhsT=xtb[:, i, 128 * st : 128 * (st + 1)],
                        rhs=w[:, i, 320 * nh : 320 * (nh + 1)],
                        start=(i == 0),
                        stop=(i == NST - 1),
                    )
                acc_sl = acc_v[:, st, 320 * nh : 320 * (nh + 1)]
                if b == 0:
                    nc.vector.tensor_copy(acc_sl, ps[:, :320])
                elif b < NSB - 1:
                    nc.vector.tensor_add(acc_sl, acc_sl, ps[:, :320])
                else:
                    nc.vector.tensor_add(
                        v_sb[:, st, 320 * nh : 320 * (nh + 1)], acc_sl, ps[:, :320]
                    )
                yield


def _attn_tail(nc, pp, v_q, po, ptsum, ones_sb, mask_sb, ci, h, pending,
               pos_of, nk, kd0, qlen):
    ps, kt = pending
    pos = pos_of[kt]
    ptile = pp.tile([128, 512], BF16, name=f"pt{ci}_{h}_{kt}", tag="pt")
    nc.scalar.activation(
        ptile[:, :qlen], ps[:, :qlen], mybir.ActivationFunctionType.Exp,
        bias=0.0, scale=SCALE,
    )
    if kt >= kd0:
        # zero out future keys: multiply by the {0,1} diagonal mask
        ptm = pp.tile([128, 512], BF16, name=f"ptm{ci}_{h}_{kt}", tag="ptm", bufs=3)
        nc.vector.tensor_mul(
            ptm[:, :qlen], ptile[:, :qlen], mask_sb[:, kt - kd0, :qlen]
        )
        ptile = ptm
    nc.tensor.matmul(
        po[:, :qlen],
        lhsT=v_q[kt // 4][:, kt % 4, 128 * h : 128 * (h + 1)],
        rhs=ptile[:, :qlen],
        start=(pos == 0), stop=(pos == nk - 1),
    )
    # denominator accumulation on DVE (off the PE): ptsum += exp tile
    if pos == 0:
        nc.vector.tensor_copy(ptsum[:, :qlen], ptile[:, :qlen])
    else:
        nc.vector.tensor_add(ptsum[:, :qlen], ptsum[:, :qlen], ptile[:, :qlen])


def _attn_chunk_gen(nc, ci, q_sb, k_q, v_q, mask_sb, ones_sb, pools, ot_b, og):
    (psA, psB, psD, pp, op_) = pools
    q0, qlen = CHUNKS[ci]
    nk = (q0 + qlen) // 128
    kd0 = q0 // 128
    order = list(range(nk))
    pos_of = {kt: i for i, kt in enumerate(order)}
    for h in range(HPC):
        qtile = q_sb[:, h, (q0 % SEQ_BLK) : (q0 % SEQ_BLK) + qlen]
        po = psB.tile([128, 512], F32, name=f"po{ci}_{h}", tag="po")
        ptsum = pp.tile([128, 512], BF16, name=f"ptsum{ci}_{h}", tag="ptsum", bufs=2)

        # 2-deep software pipeline: emit S-matmuls two tiles ahead of
        # the exp/PV consumers so PE never waits on the exp chain.
        queue = []
        for kt in order:
            ps = psA.tile([128, 512], F32, name=f"ps{ci}_{h}_{kt}", tag="ps")
            nc.tensor.matmul(
                ps[:, :qlen],
                lhsT=k_q[kt // 4][:, h, 128 * (kt % 4) : 128 * (kt % 4 + 1)],
                rhs=qtile,
                start=True,
                stop=True,
            )
            queue.append((ps, kt))
            if len(queue) > 3:
                _attn_tail(nc, pp, v_q, po, ptsum, ones_sb, mask_sb, ci, h,
                           queue.pop(0), pos_of, nk, kd0, qlen)
            yield
        for pend in queue:
            _attn_tail(nc, pp, v_q, po, ptsum, ones_sb, mask_sb, ci, h,
                       pend, pos_of, nk, kd0, qlen)
            yield

        # single ones-matmul turns the DVE-accumulated exp sums into the
        # softmax denominators for all 512 queries of this head
        pl = psD.tile([128, 512], F32, name=f"pl{ci}_{h}", tag="py")
        nc.tensor.matmul(
            pl[:, :qlen], lhsT=ones_sb[:, :], rhs=ptsum[:, :qlen],
            start=True, stop=True,
        )
        linv = op_.tile([128, 512], F32, name=f"linv{ci}_{h}", tag="linv")
        _scalar_reciprocal(nc, linv[:, :qlen], pl[:, :qlen])
        ot = op_.tile([128, 512], BF16, name=f"ot{ci}_{h}", tag="ot", bufs=2)
        nc.vector.tensor_mul(ot[:, :qlen], po[:, :qlen], linv[:, :qlen])
        nc.gpsimd.dma_start(ot_b[ci][128 * h : 128 * (h + 1), :], ot[:, :qlen])
        yield
    nc.gpsimd.collective_compute(
        "AllGather",
        mybir.AluOpType.bypass,
        replica_groups=RG,
        ins=[ot_b[ci].ap().opt()],
        outs=[og[ci].ap().opt()],
    )


def _load_wcols(nc, wcp, wot_t):
    """o_proj weights, SBUF-resident for the whole tail of the kernel."""
    wcols = {}
    for mt in range(HPC):
        for piece in range(2):
            wcol = wcp.tile(
                [128, NH // 2, 128], BF16, name=f"wo{mt}_{piece}", tag=f"wc{mt}_{piece}"
            )
            nc.scalar.dma_start(
                wcol[:, :, :],
                wot_t[mt, :, piece * (HID // 2) : (piece + 1) * (HID // 2)].rearrange(
                    "p (a b) -> p a b", a=NH // 2
                ),
            )
            wcols[(mt, piece)] = wcol
    return wcols


def _oproj_gen(nc, ogp, yp, psD, og, wcols, out, subs):
    """o_proj: full 5120 contraction per (sub-chunk, out-tile) PSUM group,
    streamed in 256-col sub-chunks; yields once per PSUM group."""
    for ci, q0, off in subs:
        halves = []
        for piece in range(2):
            ogt = ogp.tile([128, NH // 2, OSUB], BF16,
                           name=f"ogt{ci}_{off}_{piece}", tag="ogt")
            for hh in range(NH // 2):
                ht = piece * (NH // 2) + hh
                nc.sync.dma_start(
                    ogt[:, hh, :], og[ci][128 * ht : 128 * (ht + 1), off : off + OSUB]
                )
            halves.append(ogt)
        for mt in range(HPC):
            ps = psD.tile([128, 512], F32, name=f"py{ci}_{off}_{mt}", tag="py")
            for ht in range(NH):
                piece, hh = divmod(ht, NH // 2)
                nc.tensor.matmul(
                    ps[:, :OSUB],
                    lhsT=wcols[(mt, piece)][:, hh, :],
                    rhs=halves[piece][:, hh, :],
                    start=(ht == 0),
                    stop=(ht == NH - 1),
                )
            ysb = yp.tile([128, 512], F32, name=f"y{ci}_{off}_{mt}", tag="y")
            nc.vector.tensor_copy(ysb[:, :OSUB], ps[:, :OSUB])
            nc.gpsimd.dma_start(
                out[128 * mt : 128 * (mt + 1), q0 + off : q0 + off + OSUB],
                ysb[:, :OSUB],
            )
            yield


def _to_bf16(a):
    return np.asarray(a, dtype=np.float32).astype(ml_dtypes.bfloat16)


def _prep_inputs(hidden_states, W_pack_w, o_proj_w):
    xt = _to_bf16(np.ascontiguousarray(hidden_states.reshape(S, HID).T))
    # dmask[t, p, q] = 1 where key (128t + p) may be attended by query q
    # of the diagonal region (q >= 128t + p), 0 otherwise
    dmask = np.zeros((4, 128, 512), dtype=np.float32)
    for t in range(4):
        for p in range(128):
            k = 128 * t + p
            dmask[t, p, min(k, 512):] = 1.0
    dmask = dmask.astype(ml_dtypes.bfloat16)
    ones = np.ones((128, 128), dtype=ml_dtypes.bfloat16)
    in_maps = []
    for c in range(N_CORES):
        r0 = DPC * c
        # wot_t[mt][p][ht*128+c2] = o_proj_w[r0 + 128 mt + c2, 128 ht + p]
        woc = o_proj_w[r0 : r0 + DPC, :]          # [640 out, 5120 in]
        wot_t = np.ascontiguousarray(
            woc.reshape(HPC, 128, NH, 128).transpose(0, 3, 2, 1).reshape(HPC, 128, HID)
        )
        in_maps.append(
            {
                "xt": xt,
                "wqt": _to_bf16(np.ascontiguousarray(W_pack_w[r0 : r0 + DPC, :].T)),
                "wkt": _to_bf16(np.ascontiguousarray(W_pack_w[HID + r0 : HID + r0 + DPC, :].T)),
                "wvt": _to_bf16(np.ascontiguousarray(W_pack_w[2 * HID + r0 : 2 * HID + r0 + DPC, :].T)),
                "wot_t": _to_bf16(wot_t),
                "dmask": dmask,
                "ones": ones,
            }
        )
    return in_maps


def run(hidden_states, W_pack_w, o_proj_w, trace=False):
    global _graph_cache
    if _graph_cache is None:
        _graph_cache = _build_graph()
    nc = _graph_cache
    in_maps = _prep_inputs(hidden_states, W_pack_w, o_proj_w)
    res = run_bass_kernel_spmd(nc, in_maps, list(range(N_CORES)), trace=trace)
    y = np.concatenate([res.results[c]["out"].T for c in range(N_CORES)], axis=1)
    return y.reshape(1, S, HID), res


def kernel(
    hidden_states,
    W_pack_w,
    o_proj_w,
    k_cache=None,
    v_cache=None,
    input_pos=None,
    attention_mask=None,
    **_unused,
):
    hidden_states = np.asarray(hidden_states, dtype=np.float32)
    W_pack_w = np.asarray(W_pack_w, dtype=np.float32)
    o_proj_w = np.asarray(o_proj_w, dtype=np.float32)
    y, _ = run(hidden_states, W_pack_w, o_proj_w, trace=False)
    return y
